# revision 4
# baseline (speedup 1.0000x reference)
"""Trainium2 Bass kernel for NaiveMHLA (nn_NaiveMHLA_876173328957).

Sharding: 8 cores = 2 batch groups x 4 head-groups. Core c handles batch
b = c//4 and heads [4g, 4g+4) with g = c%4. The W_o-absorbed v_eff maps
each head to a disjoint slice of the output channels, so the output is
column-parallel over heads: no collective needed. Latent c_kv is computed
per batch (replicated within the 4-core batch group).

All matmuls run in float32r (TF32-like, 1 cycle/row on TRN2 vs 4 for
fp32). Attention is computed in a transposed layout (scores with the key
index on partitions, queries on the free axis) so no transposes are
needed in the attention inner loop; the final y comes out as yT and is
transposed on the host during unsharding. Causal masking skips
fully-masked key tiles and adds a -BIG additive mask on diagonal tiles
before exp (softmax without max-subtraction: logits are O(1) here,
mathematically identical).
"""

import numpy as np
from contextlib import ExitStack

import concourse.bass as bass
import concourse.tile as tile
from concourse import bacc, mybir
from concourse.bass_utils import run_bass_kernel_spmd
from concourse.masks import make_identity

F32 = mybir.dt.float32
F32R = mybir.dt.float32r

B, T, C = 2, 2048, 2048
LQ, LKV = 512, 512
NH, HS = 16, 128
NCORES = 8
HPC = 4           # heads per core
TCH = 512         # t-chunk (query chunk)
NTC = T // TCH    # 4
NCB = C // 128    # 16 c-blocks
MASK_NEG = -30000.0
SCALE = 1.0 / np.sqrt(np.float32(HS))


def _round_f32r(a: np.ndarray) -> np.ndarray:
    """Round fp32 to the fp32r grid (RNE to 11-bit mantissa) on the host."""
    a = np.ascontiguousarray(a, dtype=np.float32)
    u = a.view(np.uint32).copy()
    u += 0x800 + ((u >> 12) & 1)
    u &= np.uint32(0xFFFFF000)
    return u.view(np.float32)


def build_masks() -> np.ndarray:
    """(4, 128, 512) additive masks for the 4 diagonal k-tile positions.

    In transposed layout tile (128 k, 512 q) with k0 = q0 + p*128:
    allowed iff q >= p*128 + k (within-tile indices)."""
    m = np.zeros((4, 128, TCH), dtype=np.float32)
    kk = np.arange(128)[:, None]
    qq = np.arange(TCH)[None, :]
    for p in range(4):
        m[p] = np.where(qq >= p * 128 + kk, 0.0, MASK_NEG)
    return m


def build_program(reps: int = 1):
    nc = bacc.Bacc("TRN2", target_bir_lowering=False, debug=False,
                   num_devices=NCORES)
    d_xT = nc.dram_tensor("xT", [C, T], F32R, kind="ExternalInput").ap()
    d_wdqT = nc.dram_tensor("wdqT", [C, LQ], F32R, kind="ExternalInput").ap()
    d_wdkvT = nc.dram_tensor("wdkvT", [C, LKV], F32R, kind="ExternalInput").ap()
    d_wuq = nc.dram_tensor("wuq", [C, LQ], F32R, kind="ExternalInput").ap()
    d_wuk = nc.dram_tensor("wuk", [C, LKV], F32R, kind="ExternalInput").ap()
    d_wdqg = nc.dram_tensor("wdqg", [LQ, 512], F32R, kind="ExternalInput").ap()
    d_wuqgT = nc.dram_tensor("wuqgT", [LQ, 512], F32R, kind="ExternalInput").ap()
    d_wuv = nc.dram_tensor("wuv", [C, LKV], F32R, kind="ExternalInput").ap()
    d_wogT = nc.dram_tensor("wogT", [C, 512], F32R, kind="ExternalInput").ap()
    d_masks = nc.dram_tensor("masks", [4, 128, TCH], F32, kind="ExternalInput").ap()
    d_yT = nc.dram_tensor("yT", [HPC * HS, T], F32, kind="ExternalOutput").ap()
    d_ckv = nc.dram_tensor("ckv", [T, LKV], F32, kind="ExternalOutput").ap()

    with tile.TileContext(nc) as tc:
        with ExitStack() as ctx:
            # ---------------- pools ----------------
            persist = ctx.enter_context(tc.tile_pool(name="persist", bufs=1))
            stream = ctx.enter_context(tc.tile_pool(name="stream", bufs=3))
            xtp = ctx.enter_context(tc.tile_pool(name="xtp", bufs=17))
            chunk = ctx.enter_context(tc.tile_pool(name="chunk", bufs=2))
            expp = ctx.enter_context(tc.tile_pool(name="expp", bufs=3))
            outp = ctx.enter_context(tc.tile_pool(name="outp", bufs=2))
            small = ctx.enter_context(tc.tile_pool(name="small", bufs=1))
            psum = ctx.enter_context(tc.tile_pool(name="psum", bufs=5, space="PSUM"))
            psc = ctx.enter_context(tc.tile_pool(name="psc", bufs=2, space="PSUM"))
            prs = ctx.enter_context(tc.tile_pool(name="prs", bufs=1, space="PSUM"))

            # ---------------- persistent tiles ----------------
            ckvT_sb = persist.tile([128, LKV // 128, T], F32R)   # (l, t): 32KB/p
            ckv_sb = persist.tile([128, T // 128, LKV], F32R)    # (t, l): 32KB/p
            kg_sb = persist.tile([128, HPC, LKV], F32R)          # K_g (d, j, l)
            vg_sb = persist.tile([128, LKV // 128, 512], F32R)   # V_g (l, lt, c')
            mask_sb = persist.tile([128, 4, TCH], F32)
            wuqgT_sb = persist.tile([128, 4, 512], F32R)
            ones_f = persist.tile([128, 1], F32)
            ones_r = persist.tile([128, 1], F32R)
            ones1_f = persist.tile([1, 128], F32)
            ones1_r = persist.tile([1, 128], F32R)
            ident_f = persist.tile([128, 128], F32)

            nc.sync.dma_start(mask_sb[:], d_masks.rearrange("m p n -> p m n"))
            nc.sync.dma_start(wuqgT_sb[:],
                              d_wuqgT.rearrange("(lb p) c -> p lb c", p=128))
            nc.vector.memset(ones_f[:], 1.0)
            nc.scalar.copy(ones_r[:], ones_f[:])
            nc.vector.memset(ones1_f[:], 1.0)
            nc.scalar.copy(ones1_r[:], ones1_f[:])
            make_identity(nc, ident_f[:])

            def phase0():
                if True:
                    wdqg_sb = chunk.tile([128, 4, 512], F32R, name="wdqg_sb",
                                         tag="cq", bufs=1)
                    nc.sync.dma_start(
                        wdqg_sb[:], d_wdqg.rearrange("(lb p) c -> p lb c", p=128))
                    s_ps = [psum.tile([128, LKV], F32, name=f"s_ps{i}", tag="acc")
                            for i in range(4)]
                    for cb in range(NCB):
                        wq_c = stream.tile([128, LQ], F32R, name="wq_c", tag="wa")
                        wk_c = stream.tile([128, LKV], F32R, name="wk_c", tag="wb")
                        nc.sync.dma_start(wq_c[:], d_wuq[cb * 128:(cb + 1) * 128, :])
                        nc.sync.dma_start(wk_c[:], d_wuk[cb * 128:(cb + 1) * 128, :])
                        for lt in range(4):
                            nc.tensor.matmul(s_ps[lt][:],
                                             wq_c[:, lt * 128:(lt + 1) * 128],
                                             wk_c[:],
                                             start=(cb == 0), stop=(cb == NCB - 1))
                    s_sb = chunk.tile([128, 4, LKV], F32R, name="s_sb", tag="a")
                    for lt in range(4):
                        # fold in 1/sqrt(hs)
                        nc.scalar.mul(s_sb[:, lt, :], s_ps[lt][:], float(SCALE))

                    # K_g = wdqg.T @ S  -> (c'=j*128+d, l)
                    for j in range(HPC):
                        kg_ps = psum.tile([128, LKV], F32, name="kg_ps", tag="acc")
                        for lt in range(4):
                            nc.tensor.matmul(kg_ps[:],
                                             wdqg_sb[:, lt, j * 128:(j + 1) * 128],
                                             s_sb[:, lt, :],
                                             start=(lt == 0), stop=(lt == 3))
                        nc.scalar.copy(kg_sb[:, j, :], kg_ps[:])

                    # V_g = wuv.T @ wogT -> (l, c')
                    vg_ps = [psum.tile([128, 512], F32, name=f"vg_ps{i}", tag="acc")
                             for i in range(4)]
                    for cb in range(NCB):
                        wv_c = stream.tile([128, LKV], F32R, name="wv_c", tag="wa")
                        wo_c = stream.tile([128, 512], F32R, name="wo_c", tag="wb")
                        nc.sync.dma_start(wv_c[:], d_wuv[cb * 128:(cb + 1) * 128, :])
                        nc.sync.dma_start(wo_c[:], d_wogT[cb * 128:(cb + 1) * 128, :])
                        for lt in range(4):
                            nc.tensor.matmul(vg_ps[lt][:],
                                             wv_c[:, lt * 128:(lt + 1) * 128],
                                             wo_c[:],
                                             start=(cb == 0), stop=(cb == NCB - 1))
                    for lt in range(4):
                        nc.scalar.copy(vg_sb[:, lt, :], vg_ps[lt][:])

            def phase1(tc_i):
                t0 = tc_i * TCH
                xt_c = []
                for cb in range(NCB):
                    xt = xtp.tile([128, TCH], F32R, name=f"xt{cb}", tag="xt")
                    nc.sync.dma_start(
                        xt[:], d_xT[cb * 128:(cb + 1) * 128, t0:t0 + TCH])
                    xt_c.append(xt)

                # cq chunk: (lq, t); stream wdqT
                cq_ps = [psum.tile([128, TCH], F32, name=f"cq_ps{i}", tag="acc")
                         for i in range(4)]
                for cb in range(NCB):
                    wdq_c = stream.tile([128, LQ], F32R, name="wdq_c", tag="wa")
                    nc.sync.dma_start(wdq_c[:], d_wdqT[cb * 128:(cb + 1) * 128, :])
                    for lt in range(4):
                        nc.tensor.matmul(cq_ps[lt][:],
                                         wdq_c[:, lt * 128:(lt + 1) * 128],
                                         xt_c[cb][:],
                                         start=(cb == 0), stop=(cb == NCB - 1))
                cq_sb = chunk.tile([128, 4, TCH], F32R, name="cq_sb", tag="cq",
                                   bufs=1)
                for lt in range(4):
                    nc.scalar.copy(cq_sb[:, lt, :], cq_ps[lt][:])

                # ckvT chunk -> persistent (l, t); stream wdkvT
                ckvT_ps = [psum.tile([128, TCH], F32, name=f"ckvT_ps{i}", tag="acc")
                           for i in range(4)]
                for cb in range(NCB):
                    wdkv_c = stream.tile([128, LKV], F32R, name="wdkv_c", tag="wb")
                    nc.sync.dma_start(wdkv_c[:], d_wdkvT[cb * 128:(cb + 1) * 128, :])
                    for lt in range(4):
                        nc.tensor.matmul(ckvT_ps[lt][:],
                                         wdkv_c[:, lt * 128:(lt + 1) * 128],
                                         xt_c[cb][:],
                                         start=(cb == 0), stop=(cb == NCB - 1))
                for lt in range(4):
                    nc.scalar.copy(ckvT_sb[:, lt, t0:t0 + TCH], ckvT_ps[lt][:])

                # ckv (t, l): PE-transpose of ckvT (f32 view, 2 cyc/row)
                for tt in range(TCH // 128):
                    tb = tc_i * 4 + tt
                    ckv_ps = psum.tile([128, LKV], F32, name="ckv_ps", tag="acc")
                    for lt in range(4):
                        nc.tensor.matmul(
                            ckv_ps[:, lt * 128:(lt + 1) * 128],
                            ckvT_sb[:, lt, tb * 128:(tb + 1) * 128].bitcast(F32),
                            ident_f[:],
                            is_transpose=True, start=True, stop=True)
                    ckv_f = outp.tile([128, LKV], F32, name="ckv_f", tag="ckv_f")
                    nc.scalar.copy(ckv_f[:], ckv_ps[:])
                    nc.sync.dma_start(d_ckv[tb * 128:(tb + 1) * 128, :], ckv_f[:])
                    nc.scalar.copy(ckv_sb[:, tb, :], ckv_ps[:])

                # A chunk: (c', t) for this core's 4 heads
                a_sb = chunk.tile([128, 4, TCH], F32R, name="a_sb", tag="a")
                for ct in range(4):
                    a_ps = psum.tile([128, TCH], F32, name="a_ps", tag="acc")
                    for lt in range(4):
                        nc.tensor.matmul(a_ps[:],
                                         wuqgT_sb[:, lt, ct * 128:(ct + 1) * 128],
                                         cq_sb[:, lt, :],
                                         start=(lt == 0), stop=(lt == 3))
                    nc.scalar.copy(a_sb[:, ct, :], a_ps[:])
                return a_sb

            def phase2(tc_i, a_sb):
                t0 = tc_i * TCH
                nkt = 4 * (tc_i + 1)
                for j in range(HPC):
                    # q_latT (l, q)
                    ql_sb = chunk.tile([128, 4, TCH], F32R, name="ql_sb", tag="ql",
                                       bufs=1)
                    for lt in range(4):
                        ql_ps = psum.tile([128, TCH], F32, name="ql_ps", tag="acc")
                        nc.tensor.matmul(ql_ps[:],
                                         kg_sb[:, j, lt * 128:(lt + 1) * 128],
                                         a_sb[:, j, :],
                                         start=True, stop=True)
                        nc.scalar.copy(ql_sb[:, lt, :], ql_ps[:])

                    ctx_ps = [psum.tile([128, TCH], F32, name=f"ctx_ps{i}",
                                        tag="acc") for i in range(4)]
                    rs_ps = prs.tile([1, TCH], F32, name="rs_ps", tag="rs")
                    for kt in range(nkt):
                        sc_ps = psc.tile([128, TCH], F32, name="sc_ps", tag="sc")
                        for lt in range(4):
                            nc.tensor.matmul(
                                sc_ps[:],
                                ckvT_sb[:, lt, kt * 128:(kt + 1) * 128],
                                ql_sb[:, lt, :],
                                start=(lt == 0), stop=(lt == 3))
                        p = kt - 4 * tc_i
                        if p >= 0:
                            nc.vector.tensor_add(sc_ps[:], sc_ps[:],
                                                 mask_sb[:, p, :])
                        ex = expp.tile([128, TCH], F32R, name="ex", tag="ex")
                        nc.scalar.activation(ex[:], sc_ps[:],
                                             mybir.ActivationFunctionType.Exp)
                        nc.tensor.matmul(rs_ps[:], ones_r[:], ex[:],
                                         start=(kt == 0), stop=(kt == nkt - 1))
                        for lt in range(4):
                            nc.tensor.matmul(
                                ctx_ps[lt][:],
                                ckv_sb[:, kt, lt * 128:(lt + 1) * 128],
                                ex[:],
                                start=(kt == 0), stop=(kt == nkt - 1))

                    ctx_sb = chunk.tile([128, 4, TCH], F32R, name="ctx_sb",
                                        tag="ctx", bufs=1)
                    for lt in range(4):
                        nc.scalar.copy(ctx_sb[:, lt, :], ctx_ps[lt][:])

                    rc_f = small.tile([1, TCH], F32, name="rc_f", tag="rc_f")
                    nc.vector.reciprocal(rc_f[:], rs_ps[:])
                    rc_r = small.tile([1, TCH], F32R, name="rc_r", tag="rc_r")
                    nc.scalar.copy(rc_r[:], rc_f[:])
                    bc_ps = psc.tile([128, TCH], F32, name="bc_ps", tag="sc")
                    nc.tensor.matmul(bc_ps[:], ones1_r[:], rc_r[:],
                                     start=True, stop=True)
                    bc_sb = small.tile([128, TCH], F32, name="bc_sb", tag="bc",
                                       bufs=2)
                    nc.scalar.copy(bc_sb[:], bc_ps[:])

                    yt_ps = psum.tile([128, TCH], F32, name="yt_ps", tag="acc")
                    for lt in range(4):
                        nc.tensor.matmul(yt_ps[:],
                                         vg_sb[:, lt, j * 128:(j + 1) * 128],
                                         ctx_sb[:, lt, :],
                                         start=(lt == 0), stop=(lt == 3))
                    yt_sb = outp.tile([128, TCH], F32, name="yt_sb", tag="yt")
                    nc.vector.tensor_mul(yt_sb[:], yt_ps[:], bc_sb[:])
                    nc.sync.dma_start(
                        d_yT[j * 128:(j + 1) * 128, t0:t0 + TCH], yt_sb[:])

            def body(_iv=None):
                phase0()
                for tc_i in range(NTC):
                    a_sb = phase1(tc_i)
                    phase2(tc_i, a_sb)

            if reps == 1:
                body()
            else:
                with tc.For_i(0, reps, 1) as iv:
                    body(iv)

    nc.compile()
    return nc


_CACHE = {}


def _prep_in_maps(x, Wdq, Wuq, Wdkv, Wuk, Wuv, Wo):
    r = _round_f32r
    wuq = r(Wuq)
    wuk = r(Wuk)
    wuv = r(Wuv)
    wdqT = r(np.ascontiguousarray(Wdq.T))
    wdkvT = r(np.ascontiguousarray(Wdkv.T))
    masks = build_masks()
    xT = [r(np.ascontiguousarray(np.asarray(x)[b].T)) for b in range(B)]
    in_maps = []
    for c in range(NCORES):
        b, g = divmod(c, HPC)
        in_maps.append({
            "xT": xT[b],
            "wdqT": wdqT,
            "wdkvT": wdkvT,
            "wuq": wuq,
            "wuk": wuk,
            "wdqg": r(np.ascontiguousarray(np.asarray(Wdq)[:, g * 512:(g + 1) * 512])),
            "wuqgT": r(np.ascontiguousarray(np.asarray(Wuq)[g * 512:(g + 1) * 512, :].T)),
            "wuv": wuv,
            "wogT": r(np.ascontiguousarray(np.asarray(Wo)[g * 512:(g + 1) * 512, :].T)),
            "masks": masks,
        })
    return in_maps


def kernel(x, Wdq, Wuq, Wdkv, Wuk, Wuv, Wo):
    if "nc" not in _CACHE:
        _CACHE["nc"] = build_program()
    nc = _CACHE["nc"]
    in_maps = _prep_in_maps(x, Wdq, Wuq, Wdkv, Wuk, Wuv, Wo)
    res = run_bass_kernel_spmd(nc, in_maps, core_ids=list(range(NCORES)))
    y = np.empty((B, T, C), dtype=np.float32)
    ckv = np.empty((B, T, LKV), dtype=np.float32)
    for c in range(NCORES):
        b, g = divmod(c, HPC)
        y[b, :, g * 512:(g + 1) * 512] = res.results[c]["yT"].T
        if g == 0:
            ckv[b] = res.results[c]["ckv"]
    return y, ckv


# revision 22
# speedup vs baseline: 1.1463x; 1.1463x over previous
"""Trainium2 Bass kernel for NaiveMHLA (nn_NaiveMHLA_876173328957).

Sharding: 8 cores = 2 batch groups x 4 head-groups. Core c handles batch
b = c//4 and heads [4g, 4g+4) with g = c%4. The W_o-absorbed v_eff maps
each head to a disjoint slice of the output channels, so the output is
column-parallel over heads: no collective needed. Latent c_kv is computed
per batch (replicated within the 4-core batch group).

All matmuls run in float32r (TF32-like, 1 cycle/row on TRN2 vs 4 for
fp32). Attention is computed in a transposed layout (scores with the key
index on partitions, queries on the free axis) so no transposes are
needed in the attention inner loop; the final y comes out as yT and is
transposed on the host during unsharding. Causal masking skips
fully-masked key tiles and adds a -BIG additive mask on diagonal tiles
before exp (softmax without max-subtraction: logits are O(1) here,
mathematically identical).
"""

import numpy as np
import ml_dtypes
from contextlib import ExitStack

import concourse.bass as bass
import concourse.tile as tile
from concourse import bacc, mybir
from concourse.bass_utils import run_bass_kernel_spmd
from concourse.masks import make_identity

F32 = mybir.dt.float32
F32R = mybir.dt.float32r
BF16 = mybir.dt.bfloat16

B, T, C = 2, 2048, 2048
LQ, LKV = 512, 512
NH, HS = 16, 128
NCORES = 8
HPC = 4           # heads per core
TCH = 512         # t-chunk (query chunk)
NTC = T // TCH    # 4
NCB = C // 128    # 16 c-blocks
MASK_NEG = -30000.0
SCALE = 1.0 / np.sqrt(np.float32(HS))


def _round_f32r(a: np.ndarray) -> np.ndarray:
    """Round fp32 to the fp32r grid (RNE to 11-bit mantissa) on the host."""
    a = np.ascontiguousarray(a, dtype=np.float32)
    u = a.view(np.uint32).copy()
    u += 0x800 + ((u >> 12) & 1)
    u &= np.uint32(0xFFFFF000)
    return u.view(np.float32)


def build_masks() -> np.ndarray:
    """(128, 512) sliding additive mask. In the transposed layout a diagonal
    k-tile p (k0 = q0 + p*128) masks scores columns qq < p*128 fully and the
    block [p*128, (p+1)*128) triangularly (allowed iff qq - p*128 >= kk);
    columns right of that are fully allowed. Layout: [384 cols of NEG | 128
    tri cols]; apply m[:, 512-w:] onto scores[:, 0:w] with w=(p+1)*128."""
    m = np.full((128, TCH), MASK_NEG, dtype=np.float32)
    kk = np.arange(128)[:, None]
    qq = np.arange(128)[None, :]
    m[:, 384:] = np.where(qq >= kk, 0.0, MASK_NEG)
    return m


def build_program(reps: int = 1):
    nc = bacc.Bacc("TRN2", target_bir_lowering=False, debug=False,
                   num_devices=NCORES)
    d_xT = nc.dram_tensor("xT", [C, T], F32R, kind="ExternalInput").ap()
    d_wdqT = nc.dram_tensor("wdqT", [C, LQ], F32R, kind="ExternalInput").ap()
    d_wdkvT = nc.dram_tensor("wdkvT", [C, LKV], F32R, kind="ExternalInput").ap()
    d_wuq = nc.dram_tensor("wuq", [C, LQ], BF16, kind="ExternalInput").ap()
    d_wuk = nc.dram_tensor("wuk", [C, LKV], BF16, kind="ExternalInput").ap()
    d_wdqg = nc.dram_tensor("wdqg", [LQ, 512], BF16, kind="ExternalInput").ap()
    d_wuqgT = nc.dram_tensor("wuqgT", [LQ, 512], BF16, kind="ExternalInput").ap()
    d_wuv = nc.dram_tensor("wuv", [C, LKV], F32R, kind="ExternalInput").ap()
    d_wogT = nc.dram_tensor("wogT", [C, 512], F32R, kind="ExternalInput").ap()
    d_masks = nc.dram_tensor("masks", [128, TCH], F32, kind="ExternalInput").ap()
    d_yT = nc.dram_tensor("yT", [HPC * HS, T], F32, kind="ExternalOutput").ap()
    d_ckv = nc.dram_tensor("ckv", [T, LKV], F32, kind="ExternalOutput").ap()

    with tile.TileContext(nc) as tc:
        with ExitStack() as ctx:
            # ---------------- pools ----------------
            persist = ctx.enter_context(tc.tile_pool(name="persist", bufs=1))
            stream = ctx.enter_context(tc.tile_pool(name="stream", bufs=6))
            xtp = ctx.enter_context(tc.tile_pool(name="xtp", bufs=1))
            chunk = ctx.enter_context(tc.tile_pool(name="chunk", bufs=2))
            expp = ctx.enter_context(tc.tile_pool(name="expp", bufs=2))
            outp = ctx.enter_context(tc.tile_pool(name="outp", bufs=2))
            small = ctx.enter_context(tc.tile_pool(name="small", bufs=1))
            psum = ctx.enter_context(tc.tile_pool(name="psum", bufs=5, space="PSUM"))
            psc = ctx.enter_context(tc.tile_pool(name="psc", bufs=2, space="PSUM"))
            prs = ctx.enter_context(tc.tile_pool(name="prs", bufs=1, space="PSUM"))

            # ---------------- persistent tiles ----------------
            ckvT_bf = persist.tile([128, LKV // 128, T], BF16)   # (l, t): 16KB/p
            ckv_sb = persist.tile([128, T // 128, LKV], F32R)    # (t, l): 32KB/p
            kg_sb = persist.tile([128, HPC, LKV], BF16)          # K_g (d, j, l)
            vg_sb = persist.tile([128, LKV // 128, 512], F32R)   # V_g (l, lt, c')
            mask_sb = persist.tile([128, TCH], F32)
            mask_r = persist.tile([128, TCH], F32R)
            wuqgT_sb = persist.tile([128, 4, 512], BF16)
            ones_f = persist.tile([128, 1], F32)
            ones_r = persist.tile([128, 1], F32R)
            ones1_f = persist.tile([1, 128], F32)
            ones1_r = persist.tile([1, 128], F32R)
            ident_f = persist.tile([128, 128], F32)
            ident_r = persist.tile([128, 128], F32R)

            nc.vector.memset(ones_f[:], 1.0)
            nc.scalar.copy(ones_r[:], ones_f[:])
            nc.vector.memset(ones1_f[:], 1.0)
            nc.scalar.copy(ones1_r[:], ones1_f[:])
            make_identity(nc, ident_f[:])
            nc.scalar.copy(ident_r[:], ident_f[:])

            def phase0(mid_hook=None):
                if True:
                    wdqg_sb = chunk.tile([128, 4, 512], BF16, name="wdqg_sb",
                                         tag="wdqg", bufs=1)
                    s_ps = [psum.tile([128, LKV], F32, name=f"s_ps{i}", tag="acc")
                            for i in range(4)]
                    for cb in range(NCB):
                        wq_c = stream.tile([128, LQ], BF16, name="wq_c", tag="wqbf")
                        wk_c = stream.tile([128, LKV], BF16, name="wk_c", tag="wkbf")
                        nc.sync.dma_start(wq_c[:], d_wuq[cb * 128:(cb + 1) * 128, :])
                        nc.sync.dma_start(wk_c[:], d_wuk[cb * 128:(cb + 1) * 128, :])
                        for lt in range(4):
                            nc.tensor.matmul(s_ps[lt][:],
                                             wq_c[:, lt * 128:(lt + 1) * 128],
                                             wk_c[:],
                                             start=(cb == 0), stop=(cb == NCB - 1))
                    s_sb = chunk.tile([128, 4, LKV], BF16, name="s_sb", tag="s", bufs=1)
                    for lt in range(4):
                        # fold in 1/sqrt(hs)
                        nc.scalar.mul(s_sb[:, lt, :], s_ps[lt][:], float(SCALE))

                    if mid_hook is not None:
                        mid_hook[0] = mid_hook[1]()
                    nc.sync.dma_start(
                        wdqg_sb[:], d_wdqg.rearrange("(lb p) c -> p lb c", p=128))
                    nc.sync.dma_start(mask_sb[:], d_masks)
                    nc.scalar.copy(mask_r[:], mask_sb[:])
                    nc.sync.dma_start(wuqgT_sb[:],
                                      d_wuqgT.rearrange("(lb p) c -> p lb c", p=128))

                    # K_g = wdqg.T @ S  -> (c'=j*128+d, l)
                    for j in range(HPC):
                        kg_ps = psum.tile([128, LKV], F32, name="kg_ps", tag="acc")
                        for lt in range(4):
                            nc.tensor.matmul(kg_ps[:],
                                             wdqg_sb[:, lt, j * 128:(j + 1) * 128],
                                             s_sb[:, lt, :],
                                             start=(lt == 0), stop=(lt == 3))
                        nc.scalar.copy(kg_sb[:, j, :], kg_ps[:])


            def vg_pass():
                # V_g = wuv.T @ wogT -> (l, c')
                vg_ps = [psum.tile([128, 512], F32, name=f"vg_ps{i}", tag="acc")
                         for i in range(4)]
                for cb in range(NCB):
                    wv_c = stream.tile([128, LKV], F32R, name="wv_c", tag="wa")
                    wo_c = stream.tile([128, 512], F32R, name="wo_c", tag="wb")
                    nc.sync.dma_start(wv_c[:], d_wuv[cb * 128:(cb + 1) * 128, :])
                    nc.sync.dma_start(wo_c[:], d_wogT[cb * 128:(cb + 1) * 128, :])
                    for lt in range(4):
                        nc.tensor.matmul(vg_ps[lt][:],
                                         wv_c[:, lt * 128:(lt + 1) * 128],
                                         wo_c[:],
                                         start=(cb == 0), stop=(cb == NCB - 1))
                for lt in range(4):
                    nc.scalar.copy(vg_sb[:, lt, :], vg_ps[lt][:])

            def load_xt(tc_i):
                t0 = tc_i * TCH
                xt_big = xtp.tile([128, NCB, TCH], F32R, name="xt_big", tag="xt")
                src = d_xT[:, t0:t0 + TCH].rearrange("(cb p) t -> p cb t", p=128)
                for q in range(4):
                    nc.sync.dma_start(xt_big[:, q * 4:(q + 1) * 4, :],
                                      src[:, q * 4:(q + 1) * 4, :])
                return xt_big

            def phase1(tc_i, xt_big):
                t0 = tc_i * TCH
                xt_c = [xt_big[:, cb, :] for cb in range(NCB)]

                # cq chunk: (lq, t); stream wdqT
                cq_ps = [psum.tile([128, TCH], F32, name=f"cq_ps{i}", tag="acc")
                         for i in range(4)]
                for cb in range(NCB):
                    wdq_c = stream.tile([128, LQ], F32R, name="wdq_c", tag="wa")
                    nc.sync.dma_start(wdq_c[:], d_wdqT[cb * 128:(cb + 1) * 128, :])
                    for lt in range(4):
                        nc.tensor.matmul(cq_ps[lt][:],
                                         wdq_c[:, lt * 128:(lt + 1) * 128],
                                         xt_c[cb],
                                         start=(cb == 0), stop=(cb == NCB - 1))
                cq_sb = chunk.tile([128, 4, TCH], BF16, name="cq_sb", tag="cq",
                                   bufs=1)
                for lt in range(4):
                    nc.vector.tensor_copy(cq_sb[:, lt, :], cq_ps[lt][:])

                # ckvT chunk -> persistent (l, t); stream wdkvT
                ckvT_ps = [psum.tile([128, TCH], F32, name=f"ckvT_ps{i}", tag="acc")
                           for i in range(4)]
                for cb in range(NCB):
                    wdkv_c = stream.tile([128, LKV], F32R, name="wdkv_c", tag="wb")
                    nc.sync.dma_start(wdkv_c[:], d_wdkvT[cb * 128:(cb + 1) * 128, :])
                    for lt in range(4):
                        nc.tensor.matmul(ckvT_ps[lt][:],
                                         wdkv_c[:, lt * 128:(lt + 1) * 128],
                                         xt_c[cb],
                                         start=(cb == 0), stop=(cb == NCB - 1))
                ckvTf = chunk.tile([128, 4, TCH], F32R, name="ckvTf", tag="ckvTf")
                for lt in range(4):
                    nc.vector.tensor_copy(ckvTf[:, lt, :], ckvT_ps[lt][:])
                    nc.scalar.copy(ckvT_bf[:, lt, t0:t0 + TCH], ckvT_ps[lt][:])

                # ckv (t, l): PE-transpose of ckvT (f32 view, 2 cyc/row)
                for tt in range(TCH // 128):
                    tb = tc_i * 4 + tt
                    ckv_ps = psum.tile([128, LKV], F32, name="ckv_ps", tag="acc")
                    for lt in range(4):
                        nc.tensor.matmul(
                            ckv_ps[:, lt * 128:(lt + 1) * 128],
                            ckvTf[:, lt, tt * 128:(tt + 1) * 128].bitcast(F32),
                            ident_f[:],
                            is_transpose=True, start=True, stop=True)
                    ckv_f = outp.tile([128, LKV], F32, name="ckv_f", tag="ckv_f")
                    nc.scalar.copy(ckv_f[:], ckv_ps[:])
                    nc.sync.dma_start(d_ckv[tb * 128:(tb + 1) * 128, :], ckv_f[:])
                    nc.vector.tensor_copy(ckv_sb[:, tb, :], ckv_ps[:])

                # A chunk: (c', t) for this core's 4 heads
                a_sb = chunk.tile([128, 4, TCH], BF16, name="a_sb", tag="a")
                for ct in range(4):
                    a_ps = psum.tile([128, TCH], F32, name="a_ps", tag="acc")
                    for lt in range(4):
                        nc.tensor.matmul(a_ps[:],
                                         wuqgT_sb[:, lt, ct * 128:(ct + 1) * 128],
                                         cq_sb[:, lt, :],
                                         start=(lt == 0), stop=(lt == 3))
                    nc.vector.tensor_copy(a_sb[:, ct, :], a_ps[:])
                return a_sb

            def phase2(tc_i, a_sb):
                t0 = tc_i * TCH
                nkt = 4 * (tc_i + 1)
                for j in range(HPC):
                    # q_latT (l, q)
                    ql_sb = chunk.tile([128, 4, TCH], BF16, name="ql_sb", tag="ql",
                                       bufs=2)
                    for lt in range(4):
                        ql_ps = psum.tile([128, TCH], F32, name="ql_ps", tag="acc")
                        nc.tensor.matmul(ql_ps[:],
                                         kg_sb[:, j, lt * 128:(lt + 1) * 128],
                                         a_sb[:, j, :],
                                         start=True, stop=True)
                        nc.vector.tensor_copy(ql_sb[:, lt, :], ql_ps[:])

                    ctx_ps = [psum.tile([128, TCH], F32, name=f"ctx_ps{i}",
                                        tag="acc") for i in range(4)]
                    rs_ps = prs.tile([1, TCH], F32, name="rs_ps", tag="rs")
                    for kt in range(nkt):
                        sc_ps = psc.tile([128, TCH], F32, name="sc_ps", tag="sc")
                        diag = kt >= 4 * tc_i
                        for lt in range(4):
                            nc.tensor.matmul(
                                sc_ps[:],
                                ckvT_bf[:, lt, kt * 128:(kt + 1) * 128],
                                ql_sb[:, lt, :],
                                start=(lt == 0), stop=(lt == 3))
                        if diag:
                            p = kt - 4 * tc_i
                            w = (p + 1) * 128
                            nc.vector.tensor_add(sc_ps[:, 0:w],
                                                 sc_ps[:, 0:w],
                                                 mask_sb[:, TCH - w:])
                        ex = expp.tile([128, TCH], F32R, name="ex", tag="ex")
                        nc.scalar.activation(ex[:], sc_ps[:],
                                             mybir.ActivationFunctionType.Exp)
                        nc.tensor.matmul(rs_ps[:], ones_r[:], ex[:],
                                         start=(kt == 0), stop=(kt == nkt - 1))
                        for lt in range(4):
                            nc.tensor.matmul(
                                ctx_ps[lt][:],
                                ckv_sb[:, kt, lt * 128:(lt + 1) * 128],
                                ex[:],
                                start=(kt == 0), stop=(kt == nkt - 1))

                    ctx_sb = chunk.tile([128, 4, TCH], F32R, name="ctx_sb",
                                        tag="ctx", bufs=1)
                    for lt in range(4):
                        if lt % 2 == 0:
                            nc.vector.tensor_copy(ctx_sb[:, lt, :], ctx_ps[lt][:])
                        else:
                            nc.scalar.copy(ctx_sb[:, lt, :], ctx_ps[lt][:])

                    rc_r = small.tile([1, TCH], F32R, name="rc_r", tag="rc_r")
                    with nc.allow_low_precision(reason="f32r recip, 11-bit mantissa ok"):
                        nc.vector.reciprocal(rc_r[:], rs_ps[:])
                    bc_ps = psc.tile([128, TCH], F32, name="bc_ps", tag="sc")
                    nc.tensor.matmul(bc_ps[:], ones1_r[:], rc_r[:],
                                     start=True, stop=True)
                    bc_sb = small.tile([128, TCH], F32, name="bc_sb", tag="bc",
                                       bufs=1)
                    nc.vector.tensor_copy(bc_sb[:], bc_ps[:])

                    yt_ps = psum.tile([128, TCH], F32, name="yt_ps", tag="acc")
                    for lt in range(4):
                        nc.tensor.matmul(yt_ps[:],
                                         vg_sb[:, lt, j * 128:(j + 1) * 128],
                                         ctx_sb[:, lt, :],
                                         start=(lt == 0), stop=(lt == 3))
                    yt_sb = outp.tile([128, TCH], F32, name="yt_sb", tag="yt")
                    nc.vector.tensor_mul(yt_sb[:], yt_ps[:], bc_sb[:])
                    nc.sync.dma_start(
                        d_yT[j * 128:(j + 1) * 128, t0:t0 + TCH], yt_sb[:])

            def body(_iv=None):
                hook = [None, lambda: load_xt(0)]
                phase0(hook)
                xt_big = hook[0]
                for tc_i in range(NTC):
                    a_sb = phase1(tc_i, xt_big)
                    if tc_i == 0:
                        vg_pass()
                    xt_big = load_xt(tc_i + 1) if tc_i + 1 < NTC else None
                    phase2(tc_i, a_sb)

            if reps == 1:
                body()
            else:
                with tc.For_i(0, reps, 1) as iv:
                    body(iv)

    nc.compile()
    return nc


_CACHE = {}


def _prep_in_maps(x, Wdq, Wuq, Wdkv, Wuk, Wuv, Wo):
    r = _round_f32r
    wuq = np.ascontiguousarray(np.asarray(Wuq)).astype(ml_dtypes.bfloat16)
    wuk = np.ascontiguousarray(np.asarray(Wuk)).astype(ml_dtypes.bfloat16)
    wuv = r(Wuv)
    wdqT = r(np.ascontiguousarray(Wdq.T))
    wdkvT = r(np.ascontiguousarray(Wdkv.T))
    masks = build_masks()
    xT = [r(np.ascontiguousarray(np.asarray(x)[b].T)) for b in range(B)]
    in_maps = []
    for c in range(NCORES):
        b, g = divmod(c, HPC)
        in_maps.append({
            "xT": xT[b],
            "wdqT": wdqT,
            "wdkvT": wdkvT,
            "wuq": wuq,
            "wuk": wuk,
            "wdqg": np.ascontiguousarray(
                np.asarray(Wdq)[:, g * 512:(g + 1) * 512]).astype(ml_dtypes.bfloat16),
            "wuqgT": np.ascontiguousarray(
                np.asarray(Wuq)[g * 512:(g + 1) * 512, :].T).astype(ml_dtypes.bfloat16),
            "wuv": wuv,
            "wogT": r(np.ascontiguousarray(np.asarray(Wo)[g * 512:(g + 1) * 512, :].T)),
            "masks": masks,
        })
    return in_maps


def kernel(x, Wdq, Wuq, Wdkv, Wuk, Wuv, Wo):
    if "nc" not in _CACHE:
        _CACHE["nc"] = build_program()
    nc = _CACHE["nc"]
    in_maps = _prep_in_maps(x, Wdq, Wuq, Wdkv, Wuk, Wuv, Wo)
    res = run_bass_kernel_spmd(nc, in_maps, core_ids=list(range(NCORES)))
    y = np.empty((B, T, C), dtype=np.float32)
    ckv = np.empty((B, T, LKV), dtype=np.float32)
    for c in range(NCORES):
        b, g = divmod(c, HPC)
        y[b, :, g * 512:(g + 1) * 512] = res.results[c]["yT"].T
        if g == 0:
            ckv[b] = res.results[c]["ckv"]
    return y, ckv
